# revision 1
# baseline (speedup 1.0000x reference)
"""GraphQLayer fused kernel for 8x trn2 NeuronCores.

Math reduction: the reference output is rank-1.
  fid = (x @ x.T)^2 ; adj = (fid >= 0.85), zero diag
  agg_scalar[i] = mean_d( (adj @ x)[i, :] ) = sum_j adj[i,j] * s[j] / 128,
  with s[j] = sum_d x[j, d].  out[i, h] = agg_scalar[i] * wsum[h] + b[h],
  wsum[h] = sum_d W[h, d].
So per core we need only:  G = x @ x.T (slab), mask = |G| >= g0
(g0 = minimal fp32 y with fl(y*y) >= 0.85, exact threshold equivalence),
then a masked weighted column-sum  agg = mask.T-reduce with weights s/128 —
done on the PE as small matmuls with the s-column as the stationary operand.
Diagonal correction and bias are folded into a host-precomputed [2048, 64]
additive term.  Sharding: row-shard the Gram over 8 cores (each core owns a
2048-column slab of x.T as the moving operand); x.T replicated.
"""

import sys
from contextlib import ExitStack

import numpy as np

sys.path.insert(0, "/opt/trn_rl_repo")

import concourse.bass as bass  # noqa: E402
import concourse.tile as tile  # noqa: E402
from concourse import bacc, mybir  # noqa: E402
from concourse.bass_utils import run_bass_kernel_spmd  # noqa: E402

N, D, H = 16384, 128, 64
NCORES = 8
MSLAB = N // NCORES          # 2048 output rows per core
MCHUNK = 512                 # m columns per pass (PSUM bank width)
NPASS = MSLAB // MCHUNK      # 4
NB = N // 128                # 128 n-blocks
THRESH = 0.85

f32 = mybir.dt.float32
f32r = mybir.dt.float32r
bf16 = mybir.dt.bfloat16
i32 = mybir.dt.int32
AOP = mybir.AluOpType
AFT = mybir.ActivationFunctionType


def _round_f32r(a: np.ndarray) -> np.ndarray:
    """Round fp32 to the fp32r-representable set (sum of two bf16 parts)."""
    import ml_dtypes
    hi = a.astype(ml_dtypes.bfloat16).astype(np.float32)
    lo = (a - hi).astype(ml_dtypes.bfloat16).astype(np.float32)
    return (hi + lo).astype(np.float32)


def _abs_threshold() -> float:
    """Minimal fp32 y such that fl(y*y) >= 0.85 (exact mask equivalence)."""
    y = np.float32(np.sqrt(np.float64(THRESH)))
    thr = np.float32(THRESH)
    while np.float32(y * y) >= thr:
        y = np.nextafter(y, np.float32(0.0))
    while np.float32(y * y) < thr:
        y = np.nextafter(y, np.float32(np.inf))
    return float(y)


GS = 2  # n-blocks per PSUM drain group


def _groups():
    gs = []
    nb = 0
    while nb < NB:
        gs.append(list(range(nb, min(nb + GS, NB))))
        nb += GS
    return gs


def _build_kernel(c_abs: float):
    nc = bacc.Bacc("TRN2", target_bir_lowering=False, debug=False,
                   num_devices=NCORES)
    xt_d = nc.dram_tensor("xt", [128, N], f32, kind="ExternalInput").ap()
    xts_d = nc.dram_tensor("xts", [128, MSLAB], f32, kind="ExternalInput").ap()
    scols_d = nc.dram_tensor("scols", [128, 2 * NB], bf16, kind="ExternalInput").ap()
    wsum_d = nc.dram_tensor("wsum", [128, H], f32, kind="ExternalInput").ap()
    addc_d = nc.dram_tensor("addc", [MSLAB, H], f32, kind="ExternalInput").ap()
    out_d = nc.dram_tensor("out", [MSLAB, H], f32, kind="ExternalOutput").ap()

    with tile.TileContext(nc) as tc:
        with ExitStack() as ctx:
            _emit(ctx, tc, out_d, xt_d, xts_d, scols_d, wsum_d, addc_d, c_abs)
    nc.compile()
    return nc


def _emit(ctx, tc, out_d, xt_d, xts_d, scols_d, wsum_d, addc_d, c_abs):
    nc = tc.nc
    xt_pool = ctx.enter_context(tc.tile_pool(name="xtp", bufs=1))
    cst_pool = ctx.enter_context(tc.tile_pool(name="cst", bufs=1))
    g_pool = ctx.enter_context(tc.tile_pool(name="gp", bufs=2, space="PSUM"))
    acc_pool = ctx.enter_context(tc.tile_pool(name="accp", bufs=1, space="PSUM"))
    outp_pool = ctx.enter_context(tc.tile_pool(name="outpp", bufs=1, space="PSUM"))
    sq_pool = ctx.enter_context(tc.tile_pool(name="sqp", bufs=4))
    msk_pool = ctx.enter_context(tc.tile_pool(name="mskp", bufs=4))
    fin_pool = ctx.enter_context(tc.tile_pool(name="finp", bufs=2))

    # --- constants / inputs resident in SBUF ---
    xts_t = cst_pool.tile([128, MSLAB], f32, tag="xts")
    nc.sync.dma_start(out=xts_t[:], in_=xts_d[:])
    scols_t = cst_pool.tile([128, 2 * NB], bf16, tag="scols")
    nc.sync.dma_start(out=scols_t[:], in_=scols_d[:])
    wsum_t = cst_pool.tile([128, H], f32, tag="wsum")
    nc.sync.dma_start(out=wsum_t[:], in_=wsum_d[:])

    xt_tiles = []
    for j in range(N // 512):
        t = xt_pool.tile([128, 512], f32, tag=f"xt{j}")
        nc.sync.dma_start(out=t[:], in_=xt_d[:, j * 512:(j + 1) * 512])
        xt_tiles.append(t)

    groups = _groups()

    for mc in range(NPASS):
        accs = [acc_pool.tile([128, MCHUNK], f32, tag=f"acc{j}",
                               name=f"acc{j}_{mc}")
                for j in range(3)]
        rhs = xts_t[:, mc * MCHUNK:(mc + 1) * MCHUNK]
        for g, nbs in enumerate(groups):
            fd = len(nbs) * MCHUNK
            gt = g_pool.tile([128, GS * MCHUNK], f32, tag="g")
            for k, nb in enumerate(nbs):
                lhs = xt_tiles[nb // 4][:, (nb % 4) * 128:(nb % 4) * 128 + 128]
                nc.tensor.matmul(out=gt[:, k * MCHUNK:(k + 1) * MCHUNK],
                                 lhsT=lhs, rhs=rhs,
                                 start=True, stop=True)
            msk = msk_pool.tile([128, GS * MCHUNK], bf16, tag="msk")
            # ACT square-drain from PSUM, alternate compare engine
            sq = sq_pool.tile([128, GS * MCHUNK], f32, tag="sq")
            nc.scalar.activation(sq[:, :fd], gt[:, :fd], AFT.Square)
            ceng = nc.gpsimd if g % 2 == 0 else nc.vector
            ceng.tensor_scalar(msk[:, :fd], sq[:, :fd],
                               THRESH, None, AOP.is_ge)
            for k, nb in enumerate(nbs):
                # lhsT = [s_hi | s_lo] exact bf16 split of s/128
                nc.tensor.matmul(out=accs[0][0:2, :],
                                 lhsT=scols_t[:, 2 * nb:2 * nb + 2],
                                 rhs=msk[:, k * MCHUNK:(k + 1) * MCHUNK],
                                 start=(nb == 0), stop=(nb == NB - 1))

        # --- finalize this m-chunk: rank-1 expansion + fused bias/diag ---
        a = fin_pool.tile([128, MCHUNK], f32, tag="aggs0",
                          name=f"aggs0_{mc}")
        nc.scalar.copy(a[0:2, :], accs[0][0:2, :])
        for sub in range(MCHUNK // 128):
            outp = outp_pool.tile([128, H], f32, tag="outp")
            lo = sub * 128
            nc.tensor.matmul(out=outp[:],
                             lhsT=a[0:2, lo:lo + 128],
                             rhs=wsum_t[0:2, :],
                             start=True, stop=True)
            row0 = mc * MCHUNK + sub * 128
            bb = fin_pool.tile([128, H], f32, tag="bb")
            nc.sync.dma_start(out=bb[:], in_=addc_d[row0:row0 + 128, :])
            ot = fin_pool.tile([128, H], f32, tag="ot")
            nc.vector.tensor_add(ot[:], outp[:], bb[:])
            nc.sync.dma_start(out=out_d[row0:row0 + 128, :], in_=ot[:])


_CACHE = {}


def kernel(x: np.ndarray, W: np.ndarray, b: np.ndarray,
           trace: bool = False, tmpdir: str | None = None):
    x = np.asarray(x, dtype=np.float32)
    W = np.asarray(W, dtype=np.float32)
    b = np.asarray(b, dtype=np.float32)

    c_abs = _abs_threshold()

    # host-side prep (cheap, O(N*D))
    xt = np.ascontiguousarray(x.T)                       # [128, N]
    import ml_dtypes
    s = (x.astype(np.float64).sum(axis=1) / 128.0).astype(np.float32)  # [N]
    s_hi = s.astype(ml_dtypes.bfloat16)
    s_lo = (s - s_hi.astype(np.float32)).astype(ml_dtypes.bfloat16)
    # scols[p, 2b + t] = (s_hi if t==0 else s_lo)[b*128 + p]
    scols = np.empty((128, 2 * NB), dtype=ml_dtypes.bfloat16)
    scols[:, 0::2] = s_hi.reshape(NB, 128).T
    scols[:, 1::2] = s_lo.reshape(NB, 128).T
    wsum1 = W.astype(np.float64).sum(axis=1).astype(np.float32)
    wsum = np.ascontiguousarray(np.broadcast_to(wsum1[None, :], (128, H))).astype(np.float32)
    # diagonal correction: subtract s_i when fl(G_ii^2) >= 0.85
    q = np.einsum("nd,nd->n", x, x, dtype=np.float64).astype(np.float32)
    dmask = (np.float32(q * q) >= np.float32(THRESH))
    corr = np.where(dmask, s, np.float32(0.0)).astype(np.float32)  # [N]
    addc = b[None, :].astype(np.float32) - np.outer(corr, wsum[0].astype(np.float32))

    if "nc" not in _CACHE:
        _CACHE["nc"] = _build_kernel(c_abs)
    nc = _CACHE["nc"]

    in_maps = []
    for c in range(NCORES):
        sl = slice(c * MSLAB, (c + 1) * MSLAB)
        in_maps.append({
            "xt": xt,
            "xts": np.ascontiguousarray(xt[:, sl]),
            "scols": scols,
            "wsum": wsum,
            "addc": np.ascontiguousarray(addc[sl]),
        })

    res = run_bass_kernel_spmd(nc, in_maps, list(range(NCORES)),
                               trace=trace, tmpdir=tmpdir)
    out = np.concatenate([r["out"] for r in res.results], axis=0)
    if trace:
        kernel.last_results = res
    return out.astype(np.float32)



# revision 20
# speedup vs baseline: 4.2699x; 4.2699x over previous
"""GraphQLayer fused kernel for 8x trn2 NeuronCores.

Math reduction: the reference output is rank-1.
  fid = (x @ x.T)^2 ; adj = (fid >= 0.85), zero diag
  agg_scalar[i] = mean_d( (adj @ x)[i, :] ) = sum_j adj[i,j] * s[j],
  with s[j] = sum_d x[j, d] / 128.  out[i, h] = agg_scalar[i] * wsum[h] + b[h],
  wsum[h] = sum_d W[h, d].
Per core: G = x @ x.T slab (fp32 PE matmul - accumulation must mirror the
fp32 reference, lower-precision Gram flips near-threshold edges and fails),
fid = fl(G^2) on ACT, mask = fid >= 0.85 on DVE, then the masked weighted
column-sum on the PE with the bf16 hi/lo split of s as the stationary
operand.  Diagonal correction and bias fold into a host-precomputed
[2048, 64] additive term.
Sharding: row-shard the Gram over 8 cores; x.T replicated.
"""

import os
import sys
from contextlib import ExitStack

import numpy as np

sys.path.insert(0, "/opt/trn_rl_repo")

import concourse.bass as bass  # noqa: E402
import concourse.tile as tile  # noqa: E402
from concourse import bacc, mybir  # noqa: E402
from concourse.bass_utils import run_bass_kernel_spmd  # noqa: E402

N, D, H = 16384, 128, 64
NCORES = 8
MSLAB = N // NCORES          # 2048 output rows per core
MCHUNK = 512                 # m columns per matmul (PSUM bank width)
NPASS = MSLAB // MCHUNK      # 4
NB = N // 128                # 128 n-blocks
NTILE = N // 2048            # 8 xt DMA tiles
THRESH = 0.85

f32 = mybir.dt.float32
f32r = mybir.dt.float32r
bf16 = mybir.dt.bfloat16
i32 = mybir.dt.int32
AOP = mybir.AluOpType
AFT = mybir.ActivationFunctionType

# "f32" (safe, reference-matching) | "f32r" (3x faster gram, risky numerics)
GRAM_MODE = os.environ.get("BASS_GRAM_MODE", "f32")


def _abs_threshold() -> float:
    """Minimal fp32 y such that fl(y*y) >= 0.85 (exact mask equivalence)."""
    y = np.float32(np.sqrt(np.float64(THRESH)))
    thr = np.float32(THRESH)
    while np.float32(y * y) >= thr:
        y = np.nextafter(y, np.float32(0.0))
    while np.float32(y * y) < thr:
        y = np.nextafter(y, np.float32(np.inf))
    return float(y)


C_ABS = _abs_threshold()


def _build_kernel():
    nc = bacc.Bacc("TRN2", target_bir_lowering=False, debug=False,
                   num_devices=NCORES)
    xdt = f32r if GRAM_MODE == "f32r" else f32
    xt_d = nc.dram_tensor("xt", [128, N], xdt, kind="ExternalInput").ap()
    xts_d = nc.dram_tensor("xts", [128, MSLAB], xdt, kind="ExternalInput").ap()
    scols_d = nc.dram_tensor("scols", [128, 2 * NB], bf16,
                             kind="ExternalInput").ap()
    wsum_d = nc.dram_tensor("wsum", [2, H], f32, kind="ExternalInput").ap()
    addc_d = nc.dram_tensor("addc", [MSLAB, H], f32, kind="ExternalInput").ap()
    out_d = nc.dram_tensor("out", [MSLAB, H], f32, kind="ExternalOutput").ap()

    with tile.TileContext(nc) as tc:
        with ExitStack() as ctx:
            _emit(ctx, tc, out_d, xt_d, xts_d, scols_d, wsum_d, addc_d)
    nc.compile()
    return nc


def _emit(ctx, tc, out_d, xt_d, xts_d, scols_d, wsum_d, addc_d):
    nc = tc.nc
    xt_pool = ctx.enter_context(tc.tile_pool(name="xtp", bufs=1))
    cst_pool = ctx.enter_context(tc.tile_pool(name="cst", bufs=1))
    g_pool = ctx.enter_context(tc.tile_pool(name="gp", bufs=2, space="PSUM"))
    acc_pool = ctx.enter_context(tc.tile_pool(name="accp", bufs=1,
                                              space="PSUM"))
    msk_pool = ctx.enter_context(tc.tile_pool(name="mskp", bufs=4))
    sq_pool = ctx.enter_context(tc.tile_pool(name="sqp", bufs=2))
    fin_pool = ctx.enter_context(tc.tile_pool(name="finp", bufs=2))
    outp_pool = ctx.enter_context(tc.tile_pool(name="outpp", bufs=2,
                                               space="PSUM"))

    # --- inputs resident in SBUF; first gram operands land first ---
    xdt = f32r if GRAM_MODE == "f32r" else f32
    # head of xt tile 0 (stationary for nb=0) and first slab chunk first,
    # so the PE starts ~1us in instead of waiting for full-tile DMAs.
    xt0h = cst_pool.tile([128, 128], xdt, tag="xt0h")
    nc.sync.dma_start(out=xt0h[:], in_=xt_d[:, 0:128])
    xts_t = cst_pool.tile([128, MSLAB], xdt, tag="xts")
    for cq in range(4):
        nc.sync.dma_start(out=xts_t[:, cq * MCHUNK:(cq + 1) * MCHUNK],
                          in_=xts_d[:, cq * MCHUNK:(cq + 1) * MCHUNK])
    scols_t = cst_pool.tile([128, 2 * NB], bf16, tag="scols")
    nc.sync.dma_start(out=scols_t[:], in_=scols_d[:])
    wsum_t = cst_pool.tile([2, H], f32, tag="wsum")
    nc.sync.dma_start(out=wsum_t[:], in_=wsum_d[:])

    xt_tiles = []
    for t in range(NTILE):
        tt = xt_pool.tile([128, 2048], xdt, tag=f"xt{t}")
        nc.sync.dma_start(out=tt[:], in_=xt_d[:, t * 2048:(t + 1) * 2048])
        xt_tiles.append(tt)

    # prefetch the additive output term so the finalize tail has no DMA wait
    addc_t = cst_pool.tile([128, (MSLAB // 128) * H], f32, tag="addc")
    for blk in range(MSLAB // 128):
        nc.sync.dma_start(out=addc_t[:, blk * H:(blk + 1) * H],
                          in_=addc_d[blk * 128:(blk + 1) * 128, :])

    accs = [acc_pool.tile([128, MCHUNK], f32, tag=f"acc{i}", name=f"acc{i}")
            for i in range(2)]

    for nb in range(NB):
        if nb == 0:
            stat = xt0h[:]
        else:
            stat = xt_tiles[nb // 16][:, (nb % 16) * 128:(nb % 16) * 128 + 128]
        gts = []
        for half in range(2):
            gt = g_pool.tile([128, 2 * MCHUNK], f32, tag="g",
                             name=f"g{nb}_{half}")
            for k in range(2):
                mc = half * 2 + k
                rhs = xts_t[:, mc * MCHUNK:(mc + 1) * MCHUNK]
                nc.tensor.matmul(out=gt[:, k * MCHUNK:(k + 1) * MCHUNK],
                                 lhsT=stat, rhs=rhs, start=True, stop=True)
            gts.append(gt)
        msks = []
        for half in range(2):
            gt = gts[half]
            msk = msk_pool.tile([128, 2 * MCHUNK], bf16, tag="msk",
                                name=f"m{nb}_{half}")
            sq = sq_pool.tile([128, 2 * MCHUNK], f32, tag="sq",
                              name=f"sq{nb}_{half}")
            nc.scalar.activation(sq[:], gt[:], AFT.Square)
            nc.vector.tensor_scalar(msk[:], sq[:], THRESH, None,
                                    AOP.is_ge)
            msks.append(msk)
        for half in range(2):
            for k in range(2):
                mc = half * 2 + k
                po = 32 * (mc % 2)
                nc.tensor.matmul(
                    out=accs[mc // 2][po:po + 2, :],
                    lhsT=scols_t[:, 2 * nb:2 * nb + 2],
                    rhs=msks[half][:, k * MCHUNK:(k + 1) * MCHUNK],
                    start=(nb == 0), stop=(nb == NB - 1))

    # --- finalize: rank-1 expansion + fused bias/diag correction ---
    for mc in range(NPASS):
        a = fin_pool.tile([2, MCHUNK], f32, tag="aggs", name=f"aggs{mc}")
        po = 32 * (mc % 2)
        nc.scalar.copy(a[:], accs[mc // 2][po:po + 2, :])
        for sub in range(MCHUNK // 128):
            outp = outp_pool.tile([128, H], f32, tag="outp")
            lo = sub * 128
            nc.tensor.matmul(out=outp[:],
                             lhsT=a[0:2, lo:lo + 128],
                             rhs=wsum_t[0:2, :],
                             start=True, stop=True)
            row0 = mc * MCHUNK + sub * 128
            blk = mc * (MCHUNK // 128) + sub
            ot = fin_pool.tile([128, H], f32, tag="ot")
            nc.vector.tensor_add(ot[:], outp[:],
                                 addc_t[:, blk * H:(blk + 1) * H])
            nc.sync.dma_start(out=out_d[row0:row0 + 128, :], in_=ot[:])


_CACHE = {}


def kernel(x: np.ndarray, W: np.ndarray, b: np.ndarray,
           trace: bool = False, tmpdir: str | None = None):
    x = np.asarray(x, dtype=np.float32)
    W = np.asarray(W, dtype=np.float32)
    b = np.asarray(b, dtype=np.float32)

    # host-side prep (cheap, O(N*D))
    import ml_dtypes
    xt = np.ascontiguousarray(x.T)                       # [128, N]
    if GRAM_MODE == "f32r":
        hi = xt.astype(ml_dtypes.bfloat16).astype(np.float32)
        lo = (xt - hi).astype(ml_dtypes.bfloat16).astype(np.float32)
        xt = hi + lo                                     # f32r-representable
    s = (x.astype(np.float64).sum(axis=1) / 128.0).astype(np.float32)
    s_hi = s.astype(ml_dtypes.bfloat16)
    s_lo = (s - s_hi.astype(np.float32)).astype(ml_dtypes.bfloat16)
    # scols[p, 2b + t] = (s_hi if t==0 else s_lo)[b*128 + p]
    scols = np.empty((128, 2 * NB), dtype=ml_dtypes.bfloat16)
    scols[:, 0::2] = s_hi.reshape(NB, 128).T
    scols[:, 1::2] = s_lo.reshape(NB, 128).T
    wsum1 = W.astype(np.float64).sum(axis=1).astype(np.float32)
    wsum = np.zeros((2, H), dtype=np.float32)
    wsum[0] = wsum1
    wsum[1] = wsum1
    # diagonal correction: subtract s_i when fl(G_ii^2) >= 0.85
    q = np.einsum("nd,nd->n", x, x, dtype=np.float64).astype(np.float32)
    dmask = (np.float32(q * q) >= np.float32(THRESH))
    corr = np.where(dmask, s, np.float32(0.0)).astype(np.float32)
    addc = b[None, :].astype(np.float32) - np.outer(corr, wsum1)

    if "nc" not in _CACHE:
        _CACHE["nc"] = _build_kernel()
    nc = _CACHE["nc"]

    in_maps = []
    for c in range(NCORES):
        sl = slice(c * MSLAB, (c + 1) * MSLAB)
        in_maps.append({
            "xt": xt,
            "xts": np.ascontiguousarray(xt[:, sl]),
            "scols": scols,
            "wsum": wsum,
            "addc": np.ascontiguousarray(addc[sl]),
        })

    res = run_bass_kernel_spmd(nc, in_maps, list(range(NCORES)),
                               trace=trace, tmpdir=tmpdir)
    out = np.concatenate([r["out"] for r in res.results], axis=0)
    if trace:
        kernel.last_results = res
    return out.astype(np.float32)


# revision 21
# speedup vs baseline: 4.2816x; 1.0027x over previous
"""GraphQLayer fused kernel for 8x trn2 NeuronCores.

Math reduction: the reference output is rank-1.
  fid = (x @ x.T)^2 ; adj = (fid >= 0.85), zero diag
  agg_scalar[i] = mean_d( (adj @ x)[i, :] ) = sum_j adj[i,j] * s[j],
  with s[j] = sum_d x[j, d] / 128.  out[i, h] = agg_scalar[i] * wsum[h] + b[h],
  wsum[h] = sum_d W[h, d].
Per core: G = x @ x.T slab (fp32 PE matmul - accumulation must mirror the
fp32 reference, lower-precision Gram flips near-threshold edges and fails),
fid = fl(G^2) on ACT, mask = fid >= 0.85 on DVE, then the masked weighted
column-sum on the PE with the bf16 hi/lo split of s as the stationary
operand.  Diagonal correction and bias fold into a host-precomputed
[2048, 64] additive term.
Sharding: row-shard the Gram over 8 cores; x.T replicated.
"""

import os
import sys
from contextlib import ExitStack

import numpy as np

sys.path.insert(0, "/opt/trn_rl_repo")

import concourse.bass as bass  # noqa: E402
import concourse.tile as tile  # noqa: E402
from concourse import bacc, mybir  # noqa: E402
from concourse.bass_utils import run_bass_kernel_spmd  # noqa: E402

N, D, H = 16384, 128, 64
NCORES = 8
MSLAB = N // NCORES          # 2048 output rows per core
MCHUNK = 512                 # m columns per matmul (PSUM bank width)
NPASS = MSLAB // MCHUNK      # 4
NB = N // 128                # 128 n-blocks
NTILE = N // 2048            # 8 xt DMA tiles
THRESH = 0.85

f32 = mybir.dt.float32
f32r = mybir.dt.float32r
bf16 = mybir.dt.bfloat16
i32 = mybir.dt.int32
AOP = mybir.AluOpType
AFT = mybir.ActivationFunctionType

# "f32" (safe, reference-matching) | "f32r" (3x faster gram, risky numerics)
GRAM_MODE = os.environ.get("BASS_GRAM_MODE", "f32")


def _abs_threshold() -> float:
    """Minimal fp32 y such that fl(y*y) >= 0.85 (exact mask equivalence)."""
    y = np.float32(np.sqrt(np.float64(THRESH)))
    thr = np.float32(THRESH)
    while np.float32(y * y) >= thr:
        y = np.nextafter(y, np.float32(0.0))
    while np.float32(y * y) < thr:
        y = np.nextafter(y, np.float32(np.inf))
    return float(y)


C_ABS = _abs_threshold()


def _build_kernel():
    nc = bacc.Bacc("TRN2", target_bir_lowering=False, debug=False,
                   num_devices=NCORES)
    xdt = f32r if GRAM_MODE == "f32r" else f32
    xt_d = nc.dram_tensor("xt", [128, N], xdt, kind="ExternalInput").ap()
    xts_d = nc.dram_tensor("xts", [128, MSLAB], xdt, kind="ExternalInput").ap()
    scols_d = nc.dram_tensor("scols", [128, 2 * NB], bf16,
                             kind="ExternalInput").ap()
    wsum_d = nc.dram_tensor("wsum", [2, H], f32, kind="ExternalInput").ap()
    addc_d = nc.dram_tensor("addc", [MSLAB, H], f32, kind="ExternalInput").ap()
    out_d = nc.dram_tensor("out", [MSLAB, H], f32, kind="ExternalOutput").ap()

    with tile.TileContext(nc) as tc:
        with ExitStack() as ctx:
            _emit(ctx, tc, out_d, xt_d, xts_d, scols_d, wsum_d, addc_d)
    nc.compile()
    return nc


def _emit(ctx, tc, out_d, xt_d, xts_d, scols_d, wsum_d, addc_d):
    nc = tc.nc
    xt_pool = ctx.enter_context(tc.tile_pool(name="xtp", bufs=1))
    cst_pool = ctx.enter_context(tc.tile_pool(name="cst", bufs=1))
    g_pool = ctx.enter_context(tc.tile_pool(name="gp", bufs=2, space="PSUM"))
    acc_pool = ctx.enter_context(tc.tile_pool(name="accp", bufs=1,
                                              space="PSUM"))
    msk_pool = ctx.enter_context(tc.tile_pool(name="mskp", bufs=4))
    sq_pool = ctx.enter_context(tc.tile_pool(name="sqp", bufs=2))
    fin_pool = ctx.enter_context(tc.tile_pool(name="finp", bufs=2))
    outp_pool = ctx.enter_context(tc.tile_pool(name="outpp", bufs=2,
                                               space="PSUM"))

    # --- inputs resident in SBUF; first gram operands land first ---
    xdt = f32r if GRAM_MODE == "f32r" else f32
    # head of xt tile 0 (stationary for nb=0) and first slab chunk first,
    # so the PE starts ~1us in instead of waiting for full-tile DMAs.
    xt0h = cst_pool.tile([128, 128], xdt, tag="xt0h")
    nc.sync.dma_start(out=xt0h[:], in_=xt_d[:, 0:128])
    xts_t = cst_pool.tile([128, MSLAB], xdt, tag="xts")
    for cq in range(4):
        nc.sync.dma_start(out=xts_t[:, cq * MCHUNK:(cq + 1) * MCHUNK],
                          in_=xts_d[:, cq * MCHUNK:(cq + 1) * MCHUNK])
    scols_t = cst_pool.tile([128, 2 * NB], bf16, tag="scols")
    nc.sync.dma_start(out=scols_t[:], in_=scols_d[:])
    wsum_t = cst_pool.tile([2, H], f32, tag="wsum")
    nc.sync.dma_start(out=wsum_t[:], in_=wsum_d[:])

    xt_tiles = []
    for t in range(NTILE):
        tt = xt_pool.tile([128, 2048], xdt, tag=f"xt{t}")
        if t == 0:
            # chunked so early n-blocks don't wait on the full 1MB tile
            for cq in range(4):
                nc.sync.dma_start(
                    out=tt[:, cq * MCHUNK:(cq + 1) * MCHUNK],
                    in_=xt_d[:, cq * MCHUNK:(cq + 1) * MCHUNK])
        else:
            nc.sync.dma_start(out=tt[:], in_=xt_d[:, t * 2048:(t + 1) * 2048])
        xt_tiles.append(tt)

    # prefetch the additive output term so the finalize tail has no DMA wait
    addc_t = cst_pool.tile([128, (MSLAB // 128) * H], f32, tag="addc")
    for blk in range(MSLAB // 128):
        nc.sync.dma_start(out=addc_t[:, blk * H:(blk + 1) * H],
                          in_=addc_d[blk * 128:(blk + 1) * 128, :])

    accs = [acc_pool.tile([128, MCHUNK], f32, tag=f"acc{i}", name=f"acc{i}")
            for i in range(2)]

    for nb in range(NB):
        if nb == 0:
            stat = xt0h[:]
        else:
            stat = xt_tiles[nb // 16][:, (nb % 16) * 128:(nb % 16) * 128 + 128]
        gts = []
        for half in range(2):
            gt = g_pool.tile([128, 2 * MCHUNK], f32, tag="g",
                             name=f"g{nb}_{half}")
            for k in range(2):
                mc = half * 2 + k
                rhs = xts_t[:, mc * MCHUNK:(mc + 1) * MCHUNK]
                nc.tensor.matmul(out=gt[:, k * MCHUNK:(k + 1) * MCHUNK],
                                 lhsT=stat, rhs=rhs, start=True, stop=True)
            gts.append(gt)
        msks = []
        for half in range(2):
            gt = gts[half]
            msk = msk_pool.tile([128, 2 * MCHUNK], bf16, tag="msk",
                                name=f"m{nb}_{half}")
            sq = sq_pool.tile([128, 2 * MCHUNK], f32, tag="sq",
                              name=f"sq{nb}_{half}")
            nc.scalar.activation(sq[:], gt[:], AFT.Square)
            nc.vector.tensor_scalar(msk[:], sq[:], THRESH, None,
                                    AOP.is_ge)
            msks.append(msk)
        for half in range(2):
            for k in range(2):
                mc = half * 2 + k
                po = 32 * (mc % 2)
                nc.tensor.matmul(
                    out=accs[mc // 2][po:po + 2, :],
                    lhsT=scols_t[:, 2 * nb:2 * nb + 2],
                    rhs=msks[half][:, k * MCHUNK:(k + 1) * MCHUNK],
                    start=(nb == 0), stop=(nb == NB - 1))

    # --- finalize: rank-1 expansion + fused bias/diag correction ---
    for mc in range(NPASS):
        a = fin_pool.tile([2, MCHUNK], f32, tag="aggs", name=f"aggs{mc}")
        po = 32 * (mc % 2)
        nc.scalar.copy(a[:], accs[mc // 2][po:po + 2, :])
        for sub in range(MCHUNK // 128):
            outp = outp_pool.tile([128, H], f32, tag="outp")
            lo = sub * 128
            nc.tensor.matmul(out=outp[:],
                             lhsT=a[0:2, lo:lo + 128],
                             rhs=wsum_t[0:2, :],
                             start=True, stop=True)
            row0 = mc * MCHUNK + sub * 128
            blk = mc * (MCHUNK // 128) + sub
            ot = fin_pool.tile([128, H], f32, tag="ot")
            nc.vector.tensor_add(ot[:], outp[:],
                                 addc_t[:, blk * H:(blk + 1) * H])
            nc.sync.dma_start(out=out_d[row0:row0 + 128, :], in_=ot[:])


_CACHE = {}


def kernel(x: np.ndarray, W: np.ndarray, b: np.ndarray,
           trace: bool = False, tmpdir: str | None = None):
    x = np.asarray(x, dtype=np.float32)
    W = np.asarray(W, dtype=np.float32)
    b = np.asarray(b, dtype=np.float32)

    # host-side prep (cheap, O(N*D))
    import ml_dtypes
    xt = np.ascontiguousarray(x.T)                       # [128, N]
    if GRAM_MODE == "f32r":
        hi = xt.astype(ml_dtypes.bfloat16).astype(np.float32)
        lo = (xt - hi).astype(ml_dtypes.bfloat16).astype(np.float32)
        xt = hi + lo                                     # f32r-representable
    s = (x.astype(np.float64).sum(axis=1) / 128.0).astype(np.float32)
    s_hi = s.astype(ml_dtypes.bfloat16)
    s_lo = (s - s_hi.astype(np.float32)).astype(ml_dtypes.bfloat16)
    # scols[p, 2b + t] = (s_hi if t==0 else s_lo)[b*128 + p]
    scols = np.empty((128, 2 * NB), dtype=ml_dtypes.bfloat16)
    scols[:, 0::2] = s_hi.reshape(NB, 128).T
    scols[:, 1::2] = s_lo.reshape(NB, 128).T
    wsum1 = W.astype(np.float64).sum(axis=1).astype(np.float32)
    wsum = np.zeros((2, H), dtype=np.float32)
    wsum[0] = wsum1
    wsum[1] = wsum1
    # diagonal correction: subtract s_i when fl(G_ii^2) >= 0.85
    q = np.einsum("nd,nd->n", x, x, dtype=np.float64).astype(np.float32)
    dmask = (np.float32(q * q) >= np.float32(THRESH))
    corr = np.where(dmask, s, np.float32(0.0)).astype(np.float32)
    addc = b[None, :].astype(np.float32) - np.outer(corr, wsum1)

    if "nc" not in _CACHE:
        _CACHE["nc"] = _build_kernel()
    nc = _CACHE["nc"]

    in_maps = []
    for c in range(NCORES):
        sl = slice(c * MSLAB, (c + 1) * MSLAB)
        in_maps.append({
            "xt": xt,
            "xts": np.ascontiguousarray(xt[:, sl]),
            "scols": scols,
            "wsum": wsum,
            "addc": np.ascontiguousarray(addc[sl]),
        })

    res = run_bass_kernel_spmd(nc, in_maps, list(range(NCORES)),
                               trace=trace, tmpdir=tmpdir)
    out = np.concatenate([r["out"] for r in res.results], axis=0)
    if trace:
        kernel.last_results = res
    return out.astype(np.float32)
